# revision 8
# baseline (speedup 1.0000x reference)
"""ControlNorm2DLoop Trainium2 kernel.

x: [64, 256, 64, 64] f32. Per-(n,c) spatial moments over (H,W), then a
sequential EMA over the batch dim updates per-channel (m, v); each sample is
normalized with the state *before* its update.

Strategy: shard C across 8 cores (32 ch/core). Single pass over x in groups
of 4 samples -> SBUF tiles [128 = 4*32 partitions, 4096 free]. Stats via
bn_stats/bn_aggr (DVE). The EMA recurrence is linear, so the within-group
state propagation is a constant triangular matrix applied on the TensorEngine
(contracts over partitions); the cross-group carry is kept as a replicated
[128,1] state tile updated with elementwise DVE ops. Normalization is done
in-place on the x tile by the scalar engine (Identity(x*scale+bias) with
per-partition scale/bias), then DMA'd out.

PE wait discipline: walrus allows only ONE sync-wait command on a
(self-loading fp32) Matmult, so all constants arrive in a single DMA that a
warmup matmul observes once, and everything else a matmul touches (rhs
vectors, recycled PSUM slots) is produced/consumed exclusively by the DVE.
"""

import sys

if "/opt/trn_rl_repo" not in sys.path:
    sys.path.insert(0, "/opt/trn_rl_repo")

from contextlib import ExitStack

import numpy as np

AFWD = 0.999
EPS = 1e-05
N, C, H, W = 64, 256, 64, 64
NCORES = 8
CSH = C // NCORES     # 32 channels per core
G = 4                 # samples per group
NG = N // G           # 16 groups
FD = H * W            # 4096
P = G * CSH           # 128 partitions

# packed const layout (columns of the [128, 513] const tile)
COL_SCAN_M = 0
COL_SCAN_V = 128
COL_TAIL_M = 256
COL_TAIL_V = 384
COL_APOW = 512
CONST_COLS = 513


def _build_const() -> np.ndarray:
    """One [128, 513] f32 tile holding all scan matrices + A^s column.

    m_vals[(s,c)] = sum_{t<s} (1-A)A^(s-1-t) mu[(t,c)] + A^s m_state[c]
    v_vals[(s,c)] = sum_{t<s} (1-A)A^(s-1-t) w'[(t,c)] + A^s v_state[c]
      with w' = var + A*(mu-m)^2  (the (1-A) lives in the matrices)
    state'[c] = sum_t (1-A)A^(G-1-t) mu[(t,c)] + A^G state[c]
      (tail matrices replicate state' across all 4 sample slots)
    """
    A = AFWD
    k = np.zeros((P, CONST_COLS), np.float32)
    for s in range(G):
        for t in range(s):
            coef = (1 - A) * A ** (s - 1 - t)
            for c in range(CSH):
                k[t * CSH + c, COL_SCAN_M + s * CSH + c] = coef
                k[t * CSH + c, COL_SCAN_V + s * CSH + c] = coef
    for t in range(G):
        coef = (1 - A) * A ** (G - 1 - t)
        for s in range(G):
            for c in range(CSH):
                k[t * CSH + c, COL_TAIL_M + s * CSH + c] = coef
                k[t * CSH + c, COL_TAIL_V + s * CSH + c] = coef
    for s in range(G):
        k[s * CSH:(s + 1) * CSH, COL_APOW] = A ** s
    return k


_CACHE = {}


def build_nc():
    """Build (and cache) the Bass program. Same program for all 8 cores."""
    if "nc" in _CACHE:
        return _CACHE["nc"]

    import concourse.bacc as bacc
    import concourse.tile as tile
    from concourse import mybir

    f32 = mybir.dt.float32
    Alu = mybir.AluOpType
    Act = mybir.ActivationFunctionType
    AG = AFWD ** G

    nc = bacc.Bacc()
    x_d = nc.declare_dram_parameter("x", [N * CSH, FD], f32, isOutput=False)
    const_d = nc.declare_dram_parameter("consts", [P, CONST_COLS], f32,
                                        isOutput=False)
    out_d = nc.declare_dram_parameter("out", [N * CSH, FD], f32, isOutput=True)

    with tile.TileContext(nc) as tc, ExitStack() as ctx:
        const = ctx.enter_context(tc.tile_pool(name="const", bufs=1))
        xp = ctx.enter_context(tc.tile_pool(name="xp", bufs=6))
        st = ctx.enter_context(tc.tile_pool(name="st", bufs=3))
        states = ctx.enter_context(tc.tile_pool(name="states", bufs=2))
        psA = ctx.enter_context(tc.tile_pool(name="psA", bufs=2, space="PSUM"))
        psB = ctx.enter_context(tc.tile_pool(name="psB", bufs=1, space="PSUM"))

        ct = const.tile([P, CONST_COLS], f32)
        nc.sync.dma_start(out=ct, in_=const_d[:])
        lhs_scan_m = ct[:, COL_SCAN_M:COL_SCAN_M + P]
        lhs_scan_v = ct[:, COL_SCAN_V:COL_SCAN_V + P]
        lhs_tail_m = ct[:, COL_TAIL_M:COL_TAIL_M + P]
        lhs_tail_v = ct[:, COL_TAIL_V:COL_TAIL_V + P]
        apow = ct[:, COL_APOW:COL_APOW + 1]

        # PE touches the const tile once, so later matmuls carry no DMA wait.
        warm = psB.tile([P, 1], f32)
        nc.tensor.matmul(warm, lhsT=lhs_scan_m, rhs=apow, start=True, stop=True)

        # replicated per-(s,c) carry state: every sample slot holds state[c]
        m_rep = states.tile([P, 1], f32)
        nc.vector.memset(m_rep, 0.0)
        v_rep = states.tile([P, 1], f32)
        nc.vector.memset(v_rep, 1.0)

        for g in range(NG):
            xt = xp.tile([P, FD], f32)
            nc.sync.dma_start(out=xt, in_=x_d[g * P:(g + 1) * P, :])

            # per-(sample,channel) mean/var over the 4096 free elements
            bnst = st.tile([P, FD // 512, 6], f32)
            xt_chunks = xt.rearrange("p (k f) -> p k f", f=512)
            for k in range(FD // 512):
                nc.vector.bn_stats(out=bnst[:, k, :], in_=xt_chunks[:, k, :])
            mv = st.tile([P, 2], f32)
            nc.vector.bn_aggr(out=mv, in_=bnst)
            mu = mv[:, 0:1]
            var = mv[:, 1:2]

            # m_vals[(s,c)] = m_{n0+s,c}: triangular part on PE, carry on DVE
            pm = psA.tile([P, 1], f32)
            nc.tensor.matmul(pm, lhsT=lhs_scan_m, rhs=mu, start=True, stop=True)
            pmrep = psB.tile([P, 1], f32)
            nc.tensor.matmul(pmrep, lhsT=lhs_tail_m, rhs=mu, start=True,
                             stop=True)
            mc = st.tile([P, 1], f32)
            nc.vector.tensor_tensor(out=mc, in0=apow, in1=m_rep, op=Alu.mult)
            m_neg = st.tile([P, 1], f32)
            nc.vector.scalar_tensor_tensor(
                out=m_neg, in0=pm, scalar=-1.0, in1=mc,
                op0=Alu.mult, op1=Alu.subtract,
            )  # -(pm + A^s*state)

            # w' = var + A*(mu - m)^2
            d = st.tile([P, 1], f32)
            nc.vector.tensor_tensor(out=d, in0=mu, in1=m_neg, op=Alu.add)
            d2 = st.tile([P, 1], f32)
            nc.vector.tensor_tensor(out=d2, in0=d, in1=d, op=Alu.mult)
            wp = st.tile([P, 1], f32)
            nc.vector.scalar_tensor_tensor(
                out=wp, in0=d2, scalar=AFWD, in1=var, op0=Alu.mult, op1=Alu.add
            )

            # v_vals + eps, assembled straight into SBUF
            pv = psA.tile([P, 1], f32)
            nc.tensor.matmul(pv, lhsT=lhs_scan_v, rhs=wp, start=True, stop=True)
            pvrep = psB.tile([P, 1], f32)
            nc.tensor.matmul(pvrep, lhsT=lhs_tail_v, rhs=wp, start=True,
                             stop=True)
            vc = st.tile([P, 1], f32)
            nc.vector.tensor_tensor(out=vc, in0=apow, in1=v_rep, op=Alu.mult)
            ve = st.tile([P, 1], f32)
            nc.vector.scalar_tensor_tensor(
                out=ve, in0=pv, scalar=EPS, in1=vc, op0=Alu.add, op1=Alu.add
            )  # pv + eps + A^s*v_state

            # next-group replicated states (serial chain)
            new_m = states.tile([P, 1], f32)
            nc.vector.scalar_tensor_tensor(
                out=new_m, in0=m_rep, scalar=AG, in1=pmrep,
                op0=Alu.mult, op1=Alu.add,
            )
            m_rep = new_m
            new_v = states.tile([P, 1], f32)
            nc.vector.scalar_tensor_tensor(
                out=new_v, in0=v_rep, scalar=AG, in1=pvrep,
                op0=Alu.mult, op1=Alu.add,
            )
            v_rep = new_v

            # scale = 1/sqrt(v + eps); bias = -m * scale
            s0 = st.tile([P, 1], f32)
            nc.scalar.activation(out=s0, in_=ve, func=Act.Sqrt)
            sc = st.tile([P, 1], f32)
            nc.vector.reciprocal(out=sc, in_=s0)
            b = st.tile([P, 1], f32)
            nc.vector.tensor_scalar(
                out=b, in0=m_neg, scalar1=sc, scalar2=None, op0=Alu.mult
            )

            # out = x*scale + bias, in place, then store
            nc.scalar.activation(
                out=xt, in_=xt, func=Act.Identity, bias=b, scale=sc
            )
            nc.sync.dma_start(out=out_d[g * P:(g + 1) * P, :], in_=xt)

    nc.compile()
    _CACHE["nc"] = nc
    return nc


def kernel(x: np.ndarray) -> np.ndarray:
    assert x.shape == (N, C, H, W) and x.dtype == np.float32
    nc = build_nc()
    from concourse.bass_utils import run_bass_kernel_spmd

    consts = _build_const()
    in_maps = []
    for k in range(NCORES):
        shard = np.ascontiguousarray(
            x[:, k * CSH:(k + 1) * CSH]
        ).reshape(N * CSH, FD)
        in_maps.append({"x": shard, "consts": consts})

    res = run_bass_kernel_spmd(nc, in_maps, core_ids=list(range(NCORES)))
    shards = [res.results[k]["out"].reshape(N, CSH, H, W) for k in range(NCORES)]
    return np.concatenate(shards, axis=1)
